# revision 15
# baseline (speedup 1.0000x reference)
"""PSMNet-style concat cost volume on 8 Trainium2 NeuronCores.

Full op: inputs ref/tgt [B=4, C=32, H=64, W=128] f32 ->
output [B, 2C=64, D=48, H, W] f32 where
  out[b, :C,  d, h, w] = ref[b, :, h, w]      if w >= d else 0
  out[b, C:,  d, h, w] = tgt[b, :, h, w - d]  if w >= d else 0

Sharding: 8 cores = B(4) x H-halves(2). Each core handles one (b, h-half):
output 50.3 MB. Pure data movement -> HBM-write bound (~358 GB/s/core).

Per-core kernel (raw Bass, SWDGE DMAs, explicit semaphores):
SBUF partition p = q*32 + c, q in [0,4) = disparity offset within a 4-plane
batch, c = channel. Host sends ref replicated 4x over q [128, 32, 128] and
tgt as 4 replicas pre-shifted right by 48+q columns in zero-padded 180-wide
rows [128, 32, 180]. Staging batch [d0, d0+4) into one [128, 2, HL, W] tile:
  half 0 (ref): whole-tile DVE copy + per-q left-margin memset (width d0+q)
  half 1 (tgt): whole-tile DVE copy at column offset 48-d0 (zeros come along)
The per-core output is laid out [D, C, 2, HL, W], so a whole staged batch is
ONE fully-contiguous 4 MB SWDGE DMA (software descriptor generation is the
throughput limit for strided destinations); the host permutes during
assembly. Slot reuse is guarded by per-slot completion semaphores: waiting
for 16*(prior uses) equals the sem's maximum possible value at that point,
which implies every SDMA engine finished all prior reads of the slot --
exact, so staging pipelines freely ahead of the DMAs.
"""

from contextlib import ExitStack

import numpy as np

B, C, H, W, D = 4, 32, 64, 128, 48
HL = H // 2          # local H rows per core
NCORES = 8
PAD = D              # left zero-padding columns for shifted tgt replicas
TW = PAD + W + 4     # padded tgt row width (180)
ND = 4               # disparity planes per staged DMA batch
NB = D // ND
NSLOT = 3            # staging buffers

_nc_cache = None


def _build_bass(reps=1):
    import concourse.bass as bass
    import concourse.mybir as mybir

    dt = mybir.dt.float32
    nc = bass.Bass()
    ref = nc.declare_dram_parameter("ref", [ND * C, HL, W], dt, isOutput=False)
    tgt = nc.declare_dram_parameter("tgt", [ND * C, HL, TW], dt, isOutput=False)
    out = nc.declare_dram_parameter("out", [D, C, 2, HL, W], dt, isOutput=True)

    NK = NB * reps

    with ExitStack() as ctx:
        ref_rep = ctx.enter_context(nc.sbuf_tensor("ref_rep", [128, HL, W], dt))
        tgt_rep = ctx.enter_context(nc.sbuf_tensor("tgt_rep", [128, HL, TW], dt))
        st = [
            ctx.enter_context(nc.sbuf_tensor(f"st{i}", [128, 2, HL, W], dt))
            for i in range(NSLOT)
        ]
        s_in_r = ctx.enter_context(nc.semaphore("s_in_r"))
        s_in_t = ctx.enter_context(nc.semaphore("s_in_t"))
        s_v = ctx.enter_context(nc.semaphore("s_v"))
        s_s = [
            ctx.enter_context(nc.semaphore(f"s_s{m}")) for m in range(NSLOT)
        ]
        block = ctx.enter_context(nc.Block())

        @block.gpsimd
        def _(gpsimd):
            gpsimd.dma_start(out=ref_rep[:], in_=ref[:]).then_inc(s_in_r, 16)
            gpsimd.dma_start(out=tgt_rep[:], in_=tgt[:]).then_inc(s_in_t, 16)
            for k in range(NK):
                i = k % NB
                m = k % NSLOT
                gpsimd.wait_ge(s_v, k + 1)
                gpsimd.dma_start(
                    out=out[i * ND:(i + 1) * ND], in_=st[m][:]
                ).then_inc(s_s[m], 16)
            for m in range(NSLOT):
                uses = len(range(m, NK, NSLOT))
                gpsimd.wait_ge(s_s[m], 16 * uses)

        @block.vector
        def _(vector):
            vector.wait_ge(s_in_r, 16)
            for k in range(NK):
                d0 = (k % NB) * ND
                m = k % NSLOT
                if k >= NSLOT:
                    vector.wait_ge(s_s[m], 16 * (k // NSLOT))
                sm = st[m]
                nc.vector.tensor_copy(sm[:, 0], ref_rep[:])
                for q in range(ND):
                    d = d0 + q
                    if d > 0:
                        nc.vector.memset(
                            sm[q * C:(q + 1) * C, 0, :, 0:d], 0.0
                        )
                if k == 0:
                    vector.wait_ge(s_in_t, 16)
                nc.vector.tensor_copy(
                    sm[:, 1], tgt_rep[:, :, PAD - d0:PAD - d0 + W]
                ).then_inc(s_v, 1)

    return nc


def _get_nc():
    global _nc_cache
    if _nc_cache is None:
        _nc_cache = _build_bass()
    return _nc_cache


def _make_in_maps(input_1, input_2):
    input_1 = np.asarray(input_1, dtype=np.float32)
    input_2 = np.asarray(input_2, dtype=np.float32)
    in_maps = []
    for k in range(NCORES):
        b, j = divmod(k, 2)
        sl = slice(j * HL, (j + 1) * HL)
        r = input_1[b, :, sl, :]                      # [C, HL, W]
        t = input_2[b, :, sl, :]
        rrep = np.broadcast_to(r, (ND, C, HL, W)).reshape(ND * C, HL, W)
        trep = np.zeros((ND, C, HL, TW), dtype=np.float32)
        for q in range(ND):
            trep[q, :, :, PAD + q:PAD + q + W] = t
        in_maps.append({
            "ref": np.ascontiguousarray(rrep),
            "tgt": trep.reshape(ND * C, HL, TW),
        })
    return in_maps


def _assemble(results):
    full = np.empty((B, 2 * C, D, H, W), dtype=np.float32)
    for k in range(NCORES):
        b, j = divmod(k, 2)
        o = results[k]["out"]                         # [D, C, 2, HL, W]
        sl = slice(j * HL, (j + 1) * HL)
        full[b, :C, :, sl, :] = o[:, :, 0].transpose(1, 0, 2, 3)
        full[b, C:, :, sl, :] = o[:, :, 1].transpose(1, 0, 2, 3)
    return full


def kernel(input_1, input_2):
    from concourse.bass_utils import run_bass_kernel_spmd

    nc = _get_nc()
    res = run_bass_kernel_spmd(
        nc, _make_in_maps(input_1, input_2), list(range(NCORES))
    )
    return _assemble(res.results)
